# revision 16
# baseline (speedup 1.0000x reference)
"""GroupedQueryAttention on 8 Trainium2 NeuronCores (v4).

Sharding: core c = 4*b + r handles batch b (of 2) and token chunk r (512
of 2048 tokens) for Q/attention/o_proj over ALL 16 heads. K/V projections
are sharded by KV group: core r computes group g=r's K/V for all T, then
two chunk-split AllGathers across each batch's 4 cores make every core
independent for the rest of the kernel -- no output collective.

Schedule (all engines, one pass):
  - phase 1 (K/V proj) runs chunks in payload order [own,1,2,3]; each
    chunk's K^T / p-major V payload is written to DRAM immediately and
    each half-AllGather is issued as soon as its two chunks are done, so
    the collective (~2x 30us begin+transfer) overlaps phases 1-3.
  - Q-projection is emitted in head-groups of 4, interleaved with the
    attention group that consumes them: attention group g's exp-wait
    gaps on the PE absorb group g+1's Q-proj matmuls, so softmax (the
    ~140us ACT-bound exp chain) starts ~90us earlier than a sequential
    schedule.
  - the 8 MB wq stream alternates sync/scalar HWDGE queues, 6 tiles
    deep, paced so DMA triggers never head-of-line block either queue.
  - K/V/Q/o_proj unpack DMAs ride the gpsimd (SWDGE) queue behind the
    collective issues; Wo head-tiles stream there too, each gated on a
    token DVE-write so the 8 MB stream cannot run at t=0 and fight the
    x/wq streams for HBM.
  - attention outputs transpose via XBAR dma_start_transpose
    ([128,512] -> [128,4,128] blocked transpose on the sync queue, off
    the PE); phase-1 V blocks use PE transposes (their deps resolve too
    late for an in-order DMA queue).
  - o_proj is h-outer (each at-block stationary feeds 4 matmuls), bias
    is a DVE add fused into the PSUM drain, out is written fp16 (host
    upcasts).
  - PSUM: tag "big" = 2 x [128,1024] (score pairs / o_proj nb-pairs),
    tag "half" = 2 x [128,512] (K/V/Q accumulators), tag "opk" =
    2 x [128,258] paired A@V accumulators (2 x (128 out + 1 denom)).
    start=True clears a whole PSUM bank's has_written bits, so only the
    first accumulation group in a shared bank asserts it.

All matmuls fp16 (1 PE cycle/row) with fp32 PSUM accumulation (fp8
DoubleRow was tried and fails the 2e-2 gate: quantizing any of x/Wq/Wk
/Wv/A/Wo to e4m3 alone costs 3.6e-2..5.5e-2 max-rel error). Layouts
avoid transposing the big P matrix: projections produce Q^T/K^T/V^T
directly; scores are S^T = (K^T block).T @ Q^T; exp(S^T) = P^T feeds
A@V as the stationary; V carries a ones-column so the softmax
denominator falls out of the A@V matmul for free.
"""

import math
import sys

import numpy as np

sys.path.insert(0, "/opt/trn_rl_repo")

B = 2
T = 2048
D = 2048
HEADS = 16
GROUPS = 4
HD = 128  # head dim
M = HEADS // GROUPS  # heads per group = 4
SCALE = 1.0 / math.sqrt(HD)
N_CORES = 8
TCH = 512  # token chunk per core
NTCH = T // TCH  # 4
NSB = T // 128  # 16 key blocks
NKS = D // 128  # 16 contraction steps for projections
NQ = NKS // 4  # 4 quad blocks for the x stream
NNB = D // TCH  # 4 o_proj output column blocks

NWQB = 6  # wq stream depth
AG_SPLIT = 2  # number of chunk-split AllGathers
CPA = NTCH // AG_SPLIT  # chunks per AllGather

_COMPILED = {}


def _build():
    import concourse.bass as bass
    import concourse.mybir as mybir
    import concourse.tile as tile
    from concourse import bacc
    from concourse.masks import make_identity

    f16 = mybir.dt.float16
    f32 = mybir.dt.float32
    Exp = mybir.ActivationFunctionType.Exp
    Mult = mybir.AluOpType.mult
    Add = mybir.AluOpType.add

    nc = bacc.Bacc("TRN2", target_bir_lowering=False, num_devices=N_CORES)

    # x^T as (slot, quad) row-blocks of [128, 2048]; slot 0 = own chunk
    xcb_d = nc.declare_dram_parameter("xcb", [NTCH * NQ * 128, 4 * TCH], f16,
                                      isOutput=False)
    wq_d = nc.declare_dram_parameter("wq", [128, HEADS * NKS * 128], f16,
                                     isOutput=False)
    wk_d = nc.declare_dram_parameter("wk", [128, NKS * 128], f16, isOutput=False)
    wv_d = nc.declare_dram_parameter("wv", [128, NKS * 128], f16, isOutput=False)
    # by-head layout: [128, h, nb, 512]
    wo_d = nc.declare_dram_parameter("wo", [128, HEADS * NNB * TCH], f16,
                                     isOutput=False)
    bqs_d = nc.declare_dram_parameter("bqs", [128, HEADS], f32, isOutput=False)
    bks_d = nc.declare_dram_parameter("bks", [128, 1], f32, isOutput=False)
    bvs_d = nc.declare_dram_parameter("bvs", [128, 1], f32, isOutput=False)
    bob_d = nc.declare_dram_parameter("bob", [128, D], f16, isOutput=False)
    out_d = nc.declare_dram_parameter("out", [TCH, D], f16, isOutput=True)

    groups = [[0, 1, 2, 3], [4, 5, 6, 7]]

    with tile.TileContext(nc) as tc:
        with (
            tc.tile_pool(name="const", bufs=1) as const,
            tc.tile_pool(name="work", bufs=2) as work,
            tc.tile_pool(name="psum", bufs=1, space="PSUM") as psum,
            tc.tile_pool(name="dram", bufs=1, space="DRAM") as dram,
        ):
            ident = const.tile([128, 128], f16)
            make_identity(nc, ident)
            bqs = const.tile([128, HEADS], f32)
            bks = const.tile([128, 1], f32)
            bvs = const.tile([128, 1], f32)
            bob = const.tile([128, D], f16)

            wk_sb = const.tile([128, NKS, 128], f16)
            wv_sb = const.tile([128, NKS, 128], f16)
            # Wo by head; first NQ head-slots double as x_own (dead by o_proj)
            wo_sb = const.tile([128, HEADS, NNB * TCH], f16)
            x_own = wo_sb[:, 0:NQ, :]

            kt = const.tile([128, GROUPS, T], f16)  # gathered K^T
            v_sb = const.tile([128, GROUPS, NSB, 132], f16)  # gathered V + ones
            qt = const.tile([128, HEADS, TCH], f16)  # own-chunk Q^T
            at = const.tile([128, HEADS, 4, 128], f16)  # own-chunk A^T
            nc.vector.memset(v_sb[:, :, :, 128:129], 1.0)

            # initial loads, quad-interleaved so chunk 0 can start early;
            # x_own rides sync so it doesn't serialize behind wk/wv
            for q in range(NQ):
                nc.scalar.dma_start(wk_sb[:, q * 4 : (q + 1) * 4, :],
                                    wk_d[:, q * 512 : (q + 1) * 512])
                nc.scalar.dma_start(wv_sb[:, q * 4 : (q + 1) * 4, :],
                                    wv_d[:, q * 512 : (q + 1) * 512])
                nc.sync.dma_start(x_own[:, q, :],
                                  xcb_d[q * 128 : (q + 1) * 128, :])
            nc.scalar.dma_start(bks[:], bks_d[:])
            nc.scalar.dma_start(bvs[:], bvs_d[:])
            nc.scalar.dma_start(bqs[:], bqs_d[:])

            # wq stream on the scalar HWDGE queue
            wq_tiles = {}

            def issue_wq(h):
                if h >= HEADS or h in wq_tiles:
                    return
                wqh = work.tile([128, NKS * 128], f16, tag="wq", bufs=NWQB,
                                name="wqh", uniquify=True)
                eng = nc.scalar if h % 2 == 0 else nc.sync
                eng.dma_start(
                    wqh[:], wq_d[:, h * NKS * 128 : (h + 1) * NKS * 128])
                wq_tiles[h] = wqh

            issue_wq(0)
            issue_wq(1)

            # ---- phase 1: K/V projection for own group, all T ----
            kvl = {}
            kvg = {}
            for a in range(AG_SPLIT):
                kvl[a] = dram.tile([256, CPA * TCH], f16, tag=f"kvl{a}",
                                   name=f"kvl{a}")
                kvg[a] = dram.tile([4 * 256, CPA * TCH], f16, tag=f"kvg{a}",
                                   name=f"kvg{a}")

            for c in range(NTCH):
                kacc = psum.tile([128, TCH], f32, tag="half", bufs=2, name="kacc")
                vacc = psum.tile([128, TCH], f32, tag="half", bufs=2, name="vacc")
                for q in range(NQ):
                    if c == 0:
                        x4 = x_own[:, q, :]
                    else:
                        x4t = work.tile([128, 4 * TCH], f16, tag="xs", bufs=3,
                                        name="x4t")
                        nc.sync.dma_start(
                            x4t[:],
                            xcb_d[(c * NQ + q) * 128 : (c * NQ + q + 1) * 128, :],
                        )
                        x4 = x4t[:]
                    for k2 in range(4):
                        ks = q * 4 + k2
                        xb = x4[:, k2 * TCH : (k2 + 1) * TCH]
                        nc.tensor.matmul(
                            kacc[:], wk_sb[:, ks, :], xb,
                            start=(ks == 0), stop=(ks == NKS - 1),
                        )
                        nc.tensor.matmul(
                            vacc[:], wv_sb[:, ks, :], xb,
                            start=(ks == 0), stop=(ks == NKS - 1),
                        )
                a, cc = c // CPA, c % CPA
                ktc = work.tile([128, TCH], f16, tag="ktc", bufs=2, name="ktc")
                nc.vector.tensor_scalar_add(ktc[:], kacc[:], bks[:, 0:1])
                nc.scalar.dma_start(kvl[a][0:128, cc * TCH : (cc + 1) * TCH],
                                    ktc[:])
                vtc = work.tile([128, TCH], f16, tag="vtc", bufs=2, name="vtc")
                nc.vector.tensor_scalar_add(vtc[:], vacc[:], bvs[:, 0:1])
                # PE transpose per 128-block (an XBAR dma-transpose here
                # head-of-line blocks the scalar queue on chunk deps)
                vn = work.tile([128, 4, 128], f16, tag="vt", bufs=2, name="vn")
                for sb in range(4):
                    tp = psum.tile([128, 128], f16, tag="big", bufs=2, name="tp")
                    nc.tensor.transpose(tp[:], vtc[:, sb * 128 : (sb + 1) * 128],
                                        ident[:])
                    nc.vector.tensor_copy(vn[:, sb, :], tp[:])
                nc.scalar.dma_start(kvl[a][128:256, cc * TCH : (cc + 1) * TCH],
                                    vn[:])
                # cap at tile NWQB-1: deeper tiles would wait on Q-proj
                # consumption and head-of-line block the scalar queue
                issue_wq(min(2 + 2 * c, NWQB - 1))
                issue_wq(min(3 + 2 * c, NWQB - 1))
                if cc == CPA - 1:
                    nc.gpsimd.collective_compute(
                        "AllGather", mybir.AluOpType.bypass,
                        replica_groups=groups,
                        ins=[kvl[a][:]], outs=[kvg[a][:]],
                    )

            nc.scalar.dma_start(bob[:], bob_d[:])

            # unpack gathered K^T / V into SBUF on the gpsimd queue (behind
            # the AG issues; nothing latency-critical queues after them)
            for g in range(GROUPS):
                for a in range(AG_SPLIT):
                    nc.gpsimd.dma_start(
                        kt[:, g, a * CPA * TCH : (a + 1) * CPA * TCH],
                        kvg[a][g * 256 : g * 256 + 128, :],
                    )
                    nc.gpsimd.dma_start(
                        v_sb[:, g, a * CPA * 4 : (a + 1) * CPA * 4, 0:128],
                        kvg[a][g * 256 + 128 : (g + 1) * 256, :],
                    )

            # ---- phases 2+3 interleaved: Q-proj for group g's heads, then
            # attention for group g (its exp-wait gaps absorb the next
            # group's Q-proj matmuls) ----
            def qproj_head(h):
                wqh = wq_tiles[h]
                qacc = psum.tile([128, TCH], f32, tag="half", bufs=2, name="qacc")
                for ks in range(NKS):
                    nc.tensor.matmul(
                        qacc[:], wqh[:, ks * 128 : (ks + 1) * 128],
                        x_own[:, ks // 4, (ks % 4) * TCH : (ks % 4 + 1) * TCH],
                        start=(ks == 0), stop=(ks == NKS - 1),
                    )
                nc.vector.tensor_scalar(
                    qt[:, h, :], qacc[:], SCALE, bqs[:, h : h + 1],
                    op0=Mult, op1=Add,
                )
                issue_wq(h + NWQB)

            # Wo head-tiles stream on gpsimd during attention; each DMA is
            # gated on a token DVE-write fed by the head's normalize output
            # (dep-free DMAs would otherwise be scheduled at t=0 and fight
            # the x/wq streams for HBM). Heads 0..3 also wait out Q-proj's
            # x_own reads.
            wo_order = list(range(NQ, HEADS)) + list(range(NQ))
            for g in range(GROUPS):
                for hh in range(M):
                    qproj_head(g * M + hh)
                for hh in range(M):
                    h = g * M + hh
                    opk01 = psum.tile([128, 258], f32, tag="opk", bufs=2,
                                      name="opk01")
                    opk23 = psum.tile([128, 258], f32, tag="opk", bufs=2,
                                      name="opk23")
                    opks = [(opk01, 0), (opk01, 129), (opk23, 0), (opk23, 129)]
                    for sp in range(NSB // 2):
                        sps2 = psum.tile([128, 2 * TCH], f32, tag="big", bufs=2,
                                         name="sps2")
                        for j in range(2):
                            s = sp * 2 + j
                            nc.tensor.matmul(
                                sps2[:, j * TCH : (j + 1) * TCH],
                                kt[:, g, s * 128 : (s + 1) * 128], qt[:, h, :],
                                start=True, stop=True,
                            )
                        p2 = work.tile([128, 2 * TCH], f16, tag="p", bufs=4,
                                       name="p2")
                        nc.scalar.activation(p2[:], sps2[:], Exp)
                        for j in range(2):
                            s = sp * 2 + j
                            for tb in range(4):
                                opk, off = opks[tb]
                                # start=True clears the WHOLE PSUM bank's
                                # has_written bits, so only the first group
                                # in each shared bank may assert it; the
                                # off=129 group's s=0 matmul writes fresh
                                # (per-element has_written=0) with start=False
                                nc.tensor.matmul(
                                    opk[:, off : off + 129],
                                    p2[:, j * TCH + tb * 128
                                       : j * TCH + (tb + 1) * 128],
                                    v_sb[:, g, s, 0:129],
                                    start=(s == 0 and off == 0),
                                    stop=(s == NSB - 1),
                                    skip_group_check=(off != 0),
                                )
                    o_sb = work.tile([128, TCH], f16, tag="osb", bufs=2,
                                     name="o_sb")
                    for tb in range(4):
                        opk, off = opks[tb]
                        rcp = work.tile([128, 1], f32, tag="rcp", bufs=4,
                                        name="rcp")
                        nc.vector.reciprocal(rcp[:], opk[:, off + 128 : off + 129])
                        nc.vector.tensor_scalar_mul(
                            o_sb[:, tb * 128 : (tb + 1) * 128],
                            opk[:, off : off + 128], rcp[:])
                    # at[:, h, tb, :] = o_sb[:, tb*128:+128].T via XBAR
                    nc.sync.dma_start_transpose(at[:, h], o_sb[:])
                    # token gate + Wo head-tile stream (see comment above)
                    ws = wo_order[h]
                    nc.vector.tensor_copy(wo_sb[:, ws, 0:1], o_sb[:, 0:1])
                    nc.gpsimd.dma_start(
                        wo_sb[:, ws, :],
                        wo_d[:, ws * NNB * TCH : (ws + 1) * NNB * TCH],
                    )

            # ---- phase 4: o_proj for own chunk, full D ----
            for tb in range(4):
                pp01 = psum.tile([128, 2 * TCH], f32, tag="big", bufs=2,
                                 name="pp01")
                pp23 = psum.tile([128, 2 * TCH], f32, tag="big", bufs=2,
                                 name="pp23")
                pps = [(pp01, 0), (pp01, TCH), (pp23, 0), (pp23, TCH)]
                for h in range(HEADS):
                    for nb in range(NNB):
                        pp, off = pps[nb]
                        nc.tensor.matmul(
                            pp[:, off : off + TCH],
                            at[:, h, tb, :],
                            wo_sb[:, h, nb * TCH : (nb + 1) * TCH],
                            start=(h == 0), stop=(h == HEADS - 1),
                        )
                for nb in range(NNB):
                    pp, off = pps[nb]
                    ob = work.tile([128, TCH], f16, tag="ob", bufs=4, name="ob")
                    nc.vector.scalar_tensor_tensor(
                        ob[:], pp[:, off : off + TCH], 1.0,
                        bob[:, nb * TCH : (nb + 1) * TCH],
                        op0=Mult, op1=Add,
                    )
                    nc.sync.dma_start(
                        out_d[tb * 128 : (tb + 1) * 128, nb * TCH : (nb + 1) * TCH],
                        ob[:],
                    )

    nc.compile()
    return nc


def _get_nc():
    if "nc" not in _COMPILED:
        _COMPILED["nc"] = _build()
    return _COMPILED["nc"]


def kernel(x, Wq, bq, Wk, bk, Wv, bv, Wo, bo):
    from concourse.bass_utils import run_bass_kernel_spmd

    x = np.asarray(x, np.float32)
    Wq = np.asarray(Wq, np.float32)
    Wk = np.asarray(Wk, np.float32)
    Wv = np.asarray(Wv, np.float32)
    Wo = np.asarray(Wo, np.float32)
    bq = np.asarray(bq, np.float32)
    bk = np.asarray(bk, np.float32)
    bv = np.asarray(bv, np.float32)
    bo = np.asarray(bo, np.float32)

    nc = _get_nc()

    # shared across cores
    wq_h = np.ascontiguousarray(
        Wq.reshape(NKS, 128, HEADS, 128).transpose(1, 2, 0, 3).reshape(128, -1)
    ).astype(np.float16)
    # by-head o_proj layout [128, h, nb, 512]
    wo_h = np.ascontiguousarray(
        Wo.reshape(HEADS, 128, NNB, TCH).transpose(1, 0, 2, 3).reshape(128, -1)
    ).astype(np.float16)
    bqs_h = np.ascontiguousarray((bq * SCALE).reshape(HEADS, 128).T)
    bob_h = np.ascontiguousarray(np.broadcast_to(bo.astype(np.float16), (128, D)))
    # x^T per batch, pre-blocked into (chunk, quad) [128, 2048] row-blocks
    xq16 = []
    for b in range(B):
        xTb = x[b].T.astype(np.float16)  # [D, T]
        blocks = xTb.reshape(NKS, 128, NTCH, TCH).transpose(2, 0, 1, 3)
        # [chunk, ks, 128, TCH] -> quads: [chunk, quad, 128, 4*TCH]
        blocks = blocks.reshape(NTCH, NQ, 4, 128, TCH).transpose(0, 1, 3, 2, 4)
        xq16.append(np.ascontiguousarray(blocks.reshape(NTCH, NQ * 128, 4 * TCH)))

    wk_g, wv_g, bks_g, bvs_g = [], [], [], []
    for g in range(GROUPS):
        wk_g.append(
            np.ascontiguousarray(
                Wk[:, g * HD : (g + 1) * HD].reshape(NKS, 128, HD)
                .transpose(1, 0, 2).reshape(128, -1)
            ).astype(np.float16)
        )
        wv_g.append(
            np.ascontiguousarray(
                Wv[:, g * HD : (g + 1) * HD].reshape(NKS, 128, HD)
                .transpose(1, 0, 2).reshape(128, -1)
            ).astype(np.float16)
        )
        bks_g.append(np.ascontiguousarray(bk[g * HD : (g + 1) * HD].reshape(1, HD).T))
        bvs_g.append(np.ascontiguousarray(bv[g * HD : (g + 1) * HD].reshape(1, HD).T))

    in_maps = []
    for c in range(N_CORES):
        b, r = c // 4, c % 4
        order = [r] + [i for i in range(NTCH) if i != r]
        xcb = np.concatenate([xq16[b][s] for s in order], axis=0)
        in_maps.append(
            {
                "xcb": np.ascontiguousarray(xcb),
                "wq": wq_h,
                "wk": wk_g[r],
                "wv": wv_g[r],
                "wo": wo_h,
                "bqs": bqs_h,
                "bks": bks_g[r],
                "bvs": bvs_g[r],
                "bob": bob_h,
            }
        )

    res = run_bass_kernel_spmd(nc, in_maps, list(range(N_CORES)))
    _COMPILED["last_res"] = res

    out = np.empty((B, T, D), np.float32)
    for b in range(B):
        for r in range(NTCH):
            out[b, r * TCH : (r + 1) * TCH, :] = (
                res.results[4 * b + r]["out"].astype(np.float32)
            )
    return out
